# revision 70
# baseline (speedup 1.0000x reference)
"""Llama decode attention (paged KV, GQA) as a Bass/Tile kernel on 8 TRN2 cores.

Sharding: tensor-parallel by kv-head. Core c owns q heads 4c..4c+3, kv head c,
the matching W_qkv column shard, that kv-head's slice of the paged KV cache,
and the W_o row shard. Each core computes a partial [32, 4096] output; the
host sums the 8 partials (the "all-reduce") and adds b_o.

Host-side staging builds matmul-native KV layouts per core:
  - K: [128 (head dim), TOT*128 (chunk-major tokens)]  -> score matmul lhsT
  - V: [128 (token-in-chunk), TOT*132]; per chunk cols 0:128 = V rows,
    col 128 = validity (1.0 valid / 0.0 pad-or-new-token-slot), 129:132 pad.
    The validity column doubles as the softmax-denominator accumulator via a
    [tokens,1] x [tokens,4] matmul, so no masking ops are needed on device.
The new token's k/v (computed in-kernel from the QKV projection) enter
attention through one extra 32-token "chunk" (kt_new / vnew) with a
block-diagonal probability mask, so nothing is inserted into the KV tiles.
"""

import math

import numpy as np
import ml_dtypes

H = 32
KVH = 8
D = 128
HIDDEN = 4096
Q_SIZE = H * D
KV_SIZE = KVH * D
BLOCK = 16
NBLOCKS = 8192
MAXBPS = 128
MAXCTX = 2048
B = 32
NCORES = 8
GQ = H // NCORES          # q heads per core = 4
CHUNK = 128               # tokens per chunk
BPC = CHUNK // BLOCK      # blocks per chunk = 8
VW = 129                  # V chunk width: 128 D + 1 validity
WCH = 64                  # chunks per DMA window
WAVE = 16                 # chunks per exp wave
ROPE_THETA = 10000.0
SCALE = D ** -0.5
EXP_BIAS = -2.0           # exp(s*SCALE - 2): headroom vs overflow, cancels in norm

BF16 = ml_dtypes.bfloat16


def _ceil_div(a, b):
    return -(-a // b)


class _Schedule:
    """Static per-call schedule derived from context_lens/block_tables."""

    def __init__(self, context_lens, block_tables):
        ctx = np.asarray(context_lens, np.int64)
        bt = np.asarray(block_tables, np.int64)
        self.ctx = ctx
        self.bt = bt
        self.pos = ctx - 1
        self.nch = np.maximum(1, _ceil_div(ctx, CHUNK)).astype(np.int64)
        self.tot = int(self.nch.sum())
        self.chunk_seq = np.repeat(np.arange(B), self.nch)        # [tot]
        ci = np.concatenate([np.arange(n) for n in self.nch])
        self.chunk_ci = ci                                        # [tot]

        # RoPE tables at the new-token position
        half = D // 2
        inv_freq = 1.0 / (ROPE_THETA ** (np.arange(half, dtype=np.float64) / half))
        ang = self.pos[:, None].astype(np.float64) * inv_freq[None, :]
        self.cosf = np.tile(np.cos(ang).astype(np.float32), (1, 5))  # [32, 320]
        self.sinf = np.tile(np.sin(ang).astype(np.float32), (1, 5))

        # block-diagonal probability mask for the new-token chunk
        md = np.zeros((B, GQ * B), np.float32)
        for b in range(B):
            md[b, GQ * b:GQ * (b + 1)] = 1.0
        self.mdiag = md.astype(BF16)

        # per-chunk token validity [tot, 128]: g < ctx and g != pos
        g = ci[:, None] * CHUNK + np.arange(CHUNK)[None, :]
        s = self.chunk_seq[:, None]
        self.valid = ((g < ctx[s]) & (g != self.pos[s])).astype(np.float32)

        # flat gathered block list [tot*8]
        blk = []
        for b in range(B):
            blk.append(bt[b, :self.nch[b] * BPC])
        self.blocks_flat = np.concatenate(blk)


def _emit(nc, tile, mybir, sched):
    """Emit the per-core kernel (same NEFF for all cores)."""
    from concourse.masks import make_identity
    from concourse.tile import add_dep_helper

    dt = mybir.dt
    sc = sched
    TOT = sc.tot
    NWIN = _ceil_div(TOT, WCH)
    wsizes = [min(WCH, TOT - w * WCH) for w in range(NWIN)]
    wstart = np.concatenate([[0], np.cumsum(wsizes)]).astype(int)

    # ---- DRAM I/O ----
    d_ht = nc.dram_tensor("ht", [128, 32, B], dt.bfloat16, kind="ExternalInput")
    d_wq = nc.dram_tensor("wq", [128, 32, 768], dt.bfloat16, kind="ExternalInput")
    d_wo = nc.dram_tensor("wo", [128, 4, HIDDEN], dt.bfloat16, kind="ExternalInput")
    d_bq = nc.dram_tensor("bq", [1, 768], dt.bfloat16, kind="ExternalInput")
    d_trig = nc.dram_tensor("trig", [B, 640], dt.float32, kind="ExternalInput")
    d_md = nc.dram_tensor("mdiag", [B, GQ * B], dt.bfloat16, kind="ExternalInput")
    d_kg = nc.dram_tensor("kg", [128, TOT * CHUNK], dt.bfloat16, kind="ExternalInput")
    d_vg = nc.dram_tensor("vg", [128, TOT * VW], dt.bfloat16, kind="ExternalInput")
    d_out = nc.dram_tensor("out", [128, B * HIDDEN // 128], dt.float32,
                           kind="ExternalOutput")

    with tile.TileContext(nc) as tc:
        with (
            tc.tile_pool(name="const", bufs=1) as cp,
            tc.tile_pool(name="work", bufs=1) as wp,
            tc.tile_pool(name="kwp", bufs=3) as kwp,
            tc.tile_pool(name="vwp", bufs=3) as vwp,
            tc.tile_pool(name="extp", bufs=4) as extp,
            tc.tile_pool(name="pswork", bufs=1, space="PSUM") as pswork,
            tc.tile_pool(name="pssc", bufs=2, space="PSUM") as pssc,
            tc.tile_pool(name="psacc", bufs=1, space="PSUM") as psacc,
        ):
            # ---- constants in (ht + wq first; small consts ride behind) ----
            ht = cp.tile([128, 32, B], dt.bfloat16, tag="ht")
            nc.sync.dma_start(ht[:], d_ht[:])
            wq = cp.tile([128, 32, 768], dt.bfloat16, tag="wq")
            for qd in range(4):
                nc.sync.dma_start(wq[:, 8 * qd:8 * (qd + 1), :],
                                  d_wq[:, 8 * qd:8 * (qd + 1), :])
            # small consts on the scalar-engine HWDGE queue: they must not
            # occupy the sync trigger FIFO ahead of the KV windows.
            bq = cp.tile([1, 768], dt.bfloat16, tag="bq")
            nc.scalar.dma_start(bq[:], d_bq[:])
            trig = cp.tile([B, 640], dt.float32, tag="trig")
            nc.scalar.dma_start(trig[:], d_trig[:])
            cosf = trig[:, 0:320]
            sinf = trig[:, 320:640]
            mdiag = cp.tile([B, GQ * B], dt.bfloat16, tag="mdiag")
            nc.scalar.dma_start(mdiag[:], d_md[:])
            wo = cp.tile([128, 4, HIDDEN], dt.bfloat16, tag="wo")

            zrow = cp.tile([128, 384], dt.bfloat16, tag="zrow")
            nc.vector.memset(zrow[:], 0.0)
            ones1 = cp.tile([1, 128], dt.bfloat16, tag="ones1")
            nc.vector.memset(ones1[:], 1.0)
            ones32 = cp.tile([32, 1], dt.bfloat16, tag="ones32")
            nc.vector.memset(ones32[:], 1.0)
            ident = cp.tile([128, 128], dt.bfloat16, tag="ident")
            make_identity(nc, ident[:])
            ebias = cp.tile([128, 1], dt.float32, tag="ebias")
            nc.vector.memset(ebias[:], EXP_BIAS)

            # ---- PE warm-up: long fills into one PSUM tile, pipelined ----
            wu = pswork.tile([32, 384], dt.float32, tag="mm")
            for i in range(40):
                nc.tensor.matmul(wu[:], lhsT=zrow[:, 0:32], rhs=zrow[:],
                                 start=True, stop=True, skip_group_check=True)

            # ---- QKV projection: qkv[32, 768] = hT.T @ wq + bq ----
            qkv_f = wp.tile([B, 768], dt.float32, tag="qkvf")
            for hf in range(2):
                ps = pswork.tile([B, 384], dt.float32, tag="mm")
                nc.tensor.matmul(
                    ps[:, :384], lhsT=ones1[0:1, 0:B],
                    rhs=bq[0:1, 384 * hf:384 * (hf + 1)],
                    start=True, stop=False,
                )
                for ki in range(32):
                    nc.tensor.matmul(
                        ps[:, :384],
                        lhsT=ht[:, ki, :],
                        rhs=wq[:, ki, 384 * hf:384 * (hf + 1)],
                        start=False,
                        stop=(ki == 31),
                    )
                nc.scalar.copy(qkv_f[:, 384 * hf:384 * (hf + 1)], ps[:, :384])

            # ---- RoPE (free-axis rotate-half) + cast to bf16, all 5 heads ----
            qk_rope = wp.tile([B, 768], dt.bfloat16, tag="qkrope")
            qlo = qkv_f[:, 0:640].rearrange("p (h t x) -> p h t x", t=2, x=64)[:, :, 0, :]
            qhi = qkv_f[:, 0:640].rearrange("p (h t x) -> p h t x", t=2, x=64)[:, :, 1, :]
            rlo = qk_rope[:, 0:640].rearrange("p (h t x) -> p h t x", t=2, x=64)[:, :, 0, :]
            rhi = qk_rope[:, 0:640].rearrange("p (h t x) -> p h t x", t=2, x=64)[:, :, 1, :]
            c5 = cosf.rearrange("p (h x) -> p h x", x=64)
            s5 = sinf.rearrange("p (h x) -> p h x", x=64)
            t1 = wp.tile([B, 5, 64], dt.float32, tag="t1")
            t2 = wp.tile([B, 5, 64], dt.float32, tag="t2")
            nc.vector.tensor_mul(t1[:], qlo, c5)
            nc.vector.tensor_mul(t2[:], qhi, s5)
            nc.vector.tensor_sub(rlo, t1[:], t2[:])
            t3 = wp.tile([B, 5, 64], dt.float32, tag="t1")
            t4 = wp.tile([B, 5, 64], dt.float32, tag="t2")
            nc.vector.tensor_mul(t3[:], qhi, c5)
            nc.vector.tensor_mul(t4[:], qlo, s5)
            nc.vector.tensor_add(rhi, t3[:], t4[:])
            # v: plain cast [32, 128]
            vnew = wp.tile([B, 128], dt.bfloat16, tag="vnew")
            nc.vector.tensor_copy(vnew[:], qkv_f[:, 640:768])

            # ---- transpose q heads + k: qt [128, 4b+h], kt_new [128, 32] ----
            qt = wp.tile([128, GQ * B], dt.bfloat16, tag="qt")
            kt_new = wp.tile([128, B], dt.bfloat16, tag="ktnew")
            for hh in range(5):
                pst = pswork.tile([128, B], dt.bfloat16, tag="tr", bufs=4)
                nc.tensor.transpose(
                    pst[:], qk_rope[:, 128 * hh:128 * (hh + 1)], ident[:B, :B]
                )
                if hh < 4:
                    nc.scalar.copy(qt[:, hh::4], pst[:])
                else:
                    nc.scalar.copy(kt_new[:], pst[:])

            # ---- zero the attention accumulator (data=0, defined has_written) ----
            ps_acc = psacc.tile([128, 256], dt.float32, tag="acc")
            nc.tensor.matmul(ps_acc[:, 0:256],
                             lhsT=zrow[:, 0:128], rhs=zrow[:, 0:256],
                             start=True, stop=False, skip_group_check=True)

            # ---- windows: stream K/V; waves software-pipelined one deep ----
            kdmas = []
            wins = []
            for w in range(NWIN):
                c0 = int(wstart[w])
                c1 = int(wstart[w + 1])
                wsz = c1 - c0
                kwin = kwp.tile([128, CHUNK * WCH], dt.bfloat16, tag="kw")
                kd = nc.sync.dma_start(kwin[:, :CHUNK * wsz],
                                       d_kg[:, CHUNK * c0:CHUNK * c1])
                kdmas.append(kd)
                vwin = vwp.tile([128, VW * WCH], dt.bfloat16, tag="vw")
                nc.sync.dma_start(vwin[:, :VW * wsz],
                                  d_vg[:, VW * c0:VW * c1])
                wins.append((kwin, vwin, c0, c1))

            waves = []
            for kwin, vwin, c0, c1 in wins:
                for ws in range(c0, c1, WAVE):
                    waves.append((kwin, vwin, c0, ws, min(WAVE, c1 - ws)))

            exts = {}

            def emit_scores(i):
                kwin, vwin, c0, ws, n = waves[i]
                ps_sc = pssc.tile([128, 4 * WAVE], dt.float32, tag="sc",
                                  name=f"pssc{i}")
                for j in range(n):
                    ch = ws + j
                    l = ch - c0
                    b = int(sc.chunk_seq[ch])
                    nc.tensor.matmul(
                        ps_sc[:, 4 * j:4 * (j + 1)],
                        lhsT=kwin[:, CHUNK * l:CHUNK * (l + 1)],
                        rhs=qt[:, GQ * b:GQ * (b + 1)],
                        start=True, stop=True,
                    )
                ext = extp.tile([128, 4 * WAVE], dt.bfloat16, tag="ext",
                                name=f"ext{i}")
                nc.scalar.activation(
                    ext[:, :4 * n], ps_sc[:, :4 * n],
                    mybir.ActivationFunctionType.Exp,
                    bias=ebias[:], scale=SCALE,
                )
                exts[i] = ext

            emit_scores(0)
            for i in range(len(waves)):
                if i + 1 < len(waves):
                    emit_scores(i + 1)  # PE does wave i+1 scores while exp(i) runs
                kwin, vwin, c0, ws, n = waves[i]
                ext = exts.pop(i)
                # uniform runs so the PE drain/fill overlap never breaks
                for j in range(n):
                    ch = ws + j
                    l = ch - c0
                    b = int(sc.chunk_seq[ch])
                    nc.tensor.matmul(
                        ps_acc[:, 4 * b:4 * (b + 1)],
                        lhsT=vwin[:, VW * l:VW * l + 128],
                        rhs=ext[:, 4 * j:4 * (j + 1)],
                        start=False, stop=False, skip_group_check=True,
                    )
                for j in range(n):
                    ch = ws + j
                    l = ch - c0
                    b = int(sc.chunk_seq[ch])
                    nc.tensor.matmul(
                        ps_acc[0:1, 128 + 4 * b:132 + 4 * b],
                        lhsT=vwin[:, VW * l + 128:VW * l + 129],
                        rhs=ext[:, 4 * j:4 * (j + 1)],
                        start=False, stop=False, skip_group_check=True,
                    )

            # wo streams on the scalar queue, overlapping the KV stream tail
            for wn in range(4):
                wo_dma = nc.scalar.dma_start(wo[:, :, 1024 * wn:1024 * (wn + 1)],
                                             d_wo[:, :, 1024 * wn:1024 * (wn + 1)])
                dep = kdmas[max(0, NWIN - 3)]
                add_dep_helper(wo_dma.ins, dep.ins, sync=True,
                               reason="wo overlaps KV tail")

            # ---- new-token contribution (one extra 32-token chunk) ----
            ps_x = pswork.tile([B, 128], dt.float32, tag="mm")
            nc.tensor.matmul(ps_x[:], lhsT=kt_new[:], rhs=qt[:],
                             start=True, stop=True)
            extx = wp.tile([B, 128], dt.float32, tag="extx")
            nc.scalar.activation(
                extx[:], ps_x[:], mybir.ActivationFunctionType.Exp,
                bias=ebias[0:B, :], scale=SCALE,
            )
            p2 = wp.tile([B, 128], dt.bfloat16, tag="p2")
            nc.vector.tensor_mul(p2[:], extx[:], mdiag[:])
            nc.tensor.matmul(ps_acc[:, 0:128], lhsT=vnew[:], rhs=p2[:],
                             start=False, stop=True, skip_group_check=True)
            nc.tensor.matmul(ps_acc[0:1, 128:256], lhsT=ones32[:], rhs=p2[:],
                             start=False, stop=True, skip_group_check=True)

            # ---- normalize: at = attn / denom ----
            # broadcast the sums across partitions FIRST, then a
            # 128-partition-parallel reciprocal (serial chain ~2x shorter)
            sums_bf = wp.tile([1, 128], dt.bfloat16, tag="sumsbf")
            nc.scalar.copy(sums_bf[:], ps_acc[0:1, 128:256])
            ps_rb = pswork.tile([128, 128], dt.float32, tag="mm")
            nc.tensor.matmul(ps_rb[:], lhsT=ones1[0:1, :], rhs=sums_bf[0:1, :],
                             start=True, stop=True)
            rb_sb = wp.tile([128, 128], dt.float32, tag="rbsb")
            nc.vector.reciprocal(rb_sb[:], ps_rb[:])
            # head-major layout: at_hm[:, 32*h + s] = attn[:, 4*s + h]
            at_hm = wp.tile([128, 128], dt.bfloat16, tag="athm")
            nc.vector.tensor_mul(
                at_hm[:].rearrange("p (h s) -> p s h", h=4),
                ps_acc[:, 0:128].rearrange("p (s h) -> p s h", h=4),
                rb_sb[:].rearrange("p (s h) -> p s h", h=4),
            )

            # ---- O projection, transposed: outT[4096, 32] partial ----
            # lhsT = wo[:, h, 128j:128j+128] ([d, n] block), rhs = at_hm head
            # slice ([d, s]); psum groups of 4 j-blocks -> ostage [128, 1024]
            ostage = wp.tile([128, 32 * 32], dt.float32, tag="ostage")
            for g in range(8):
                ps_o = pswork.tile([128, 128], dt.float32, tag="tr", bufs=4)
                for jj in range(4):
                    j = 4 * g + jj
                    for hh in range(4):
                        nc.tensor.matmul(
                            ps_o[:, 32 * jj:32 * (jj + 1)],
                            lhsT=wo[:, hh, 128 * j:128 * (j + 1)],
                            rhs=at_hm[:, 32 * hh:32 * (hh + 1)],
                            start=(hh == 0),
                            stop=(hh == 3),
                            skip_group_check=True,
                        )
                if g % 2 == 0:
                    nc.scalar.copy(ostage[:, 128 * g:128 * (g + 1)], ps_o[:])
                else:
                    nc.vector.tensor_copy(ostage[:, 128 * g:128 * (g + 1)], ps_o[:])
                nc.sync.dma_start(d_out[:, 128 * g:128 * (g + 1)],
                                  ostage[:, 128 * g:128 * (g + 1)])

    nc.compile()
    return nc


def _build_inputs(sched, hidden_states, W_qkv, b_qkv, W_o, k_cache, v_cache):
    """Per-core input maps with host-side gather into matmul-native layouts."""
    sc = sched
    TOT = sc.tot

    hts = hidden_states.T.astype(BF16)  # [4096, 32]
    ht_in = np.ascontiguousarray(hts.reshape(32, 128, B).transpose(1, 0, 2))

    # one global gather of the needed blocks (all kv heads at once)
    KB = k_cache[sc.blocks_flat]   # [TOT*8, 16, 8, 128] fp32
    VB = v_cache[sc.blocks_flat]

    maps = []
    for c in range(NCORES):
        qr = slice(512 * c, 512 * (c + 1))
        kr = slice(Q_SIZE + 128 * c, Q_SIZE + 128 * (c + 1))
        vr = slice(Q_SIZE + KV_SIZE + 128 * c, Q_SIZE + KV_SIZE + 128 * (c + 1))
        wq_sh = np.concatenate([W_qkv[qr], W_qkv[kr], W_qkv[vr]], axis=0)  # [768, 4096]
        wq_in = np.ascontiguousarray(
            wq_sh.T.astype(BF16).reshape(32, 128, 768).transpose(1, 0, 2))
        bq_sh = np.concatenate([b_qkv[qr], b_qkv[kr], b_qkv[vr]])
        bq_in = bq_sh[None, :].astype(BF16)
        wo_in = np.ascontiguousarray(
            W_o[:, qr].T.astype(BF16).reshape(4, 128, HIDDEN).transpose(1, 0, 2))

        # K: [TOT, 128 tok, 128 D] -> [128 D, TOT*128]
        kc = KB[:, :, c, :].astype(BF16).reshape(TOT, CHUNK, D)
        kg_in = np.ascontiguousarray(
            kc.transpose(2, 0, 1).reshape(D, TOT * CHUNK))

        # V: [TOT, 128 tok, 132]
        vc = VB[:, :, c, :].reshape(TOT, CHUNK, D)
        vg = np.zeros((TOT, CHUNK, VW), np.float32)
        vg[:, :, :D] = vc * sc.valid[:, :, None]
        vg[:, :, D] = sc.valid
        vg_in = np.ascontiguousarray(
            vg.astype(BF16).transpose(1, 0, 2).reshape(CHUNK, TOT * VW))

        maps.append({
            "ht": ht_in, "wq": wq_in, "wo": wo_in, "bq": bq_in,
            "trig": np.concatenate([sc.cosf, sc.sinf], axis=1),
            "mdiag": sc.mdiag,
            "kg": kg_in, "vg": vg_in,
        })
    return maps


_TRACE = {"on": False, "result": None}


def kernel(hidden_states, W_qkv, b_qkv, W_o, b_o, k_cache, v_cache,
           block_tables, context_lens):
    import concourse.tile as tile
    import concourse.mybir as mybir
    from concourse import bacc
    from concourse.bass_utils import run_bass_kernel_spmd

    sched = _Schedule(context_lens, block_tables)
    nc = bacc.Bacc("TRN2", target_bir_lowering=False, debug=False)
    _emit(nc, tile, mybir, sched)

    in_maps = _build_inputs(sched, np.asarray(hidden_states, np.float32),
                            np.asarray(W_qkv, np.float32),
                            np.asarray(b_qkv, np.float32),
                            np.asarray(W_o, np.float32),
                            np.asarray(k_cache, np.float32),
                            np.asarray(v_cache, np.float32))

    res = run_bass_kernel_spmd(nc, in_maps, core_ids=list(range(NCORES)),
                               trace=_TRACE["on"])
    _TRACE["result"] = res

    acc = np.zeros((B, HIDDEN), np.float64)
    for c in range(NCORES):
        o128 = res.results[c]["out"].astype(np.float64)  # [128, 1024]
        # o128[p, 32*j + s] = out[s, 128*j + p]
        acc += o128.reshape(128, 32, 32).transpose(2, 1, 0).reshape(B, HIDDEN)
    acc += np.asarray(b_o, np.float64)[None, :]
    return acc.astype(np.float32)
